# revision 29
# baseline (speedup 1.0000x reference)
"""Trainium2 Bass kernel for BaselineMultiStepRNN — split-fp32r edition.

Math (per original reference, 1-based step index t = 1..T):
    h_t   = tanh(Wx x_t + Wc cap_{t-1} + Whh h_{t-1} + b_ih + b_hh)
    drop_t = fc_w h_t + fc_b
    cap_t = cap_{t-1} - drop_t ;  out[:, t-1] = cap_t

Folded form used on device (state v_t = cap_t - fc_b):
    W'  = Whh - outer(Wc, fc_w)     (removes cap's one-step feedback lag)
    pre_t = Wx x_t + Wc v_{t-3} - Wc d_{t-2} + W' h_{t-1}  (+ b - Wc*fcb via
            the ACT bias port; note v_{t-2} = v_{t-3} - fcb - d_{t-2})
    h_t  = tanh(pre_t + bias)
    d_t  = fc_w h_t
    v_t  = (v_{t-1} - fcb) - d_t          (v_0 = cap_0 - fcb, v_{-1} = cap_0)
    out[:, t-1] = v_t + fc_b

The v_{t-3}/d_{t-2} decomposition de-stresses the cross-engine feedback
chain: the v rows are written two steps before use, and the only 1-step-lag
row is a single ACT rounding copy of d out of PSUM.

Precision: every matmul runs in float32r (1 cycle/row vs fp32's 4) using a
3-term hi/lo mantissa split that recovers fp32-grade accuracy.  TRN2's f32r
stores 11 mantissa bits (round-to-nearest on engine writes; operands with
<=11 mantissa bits pass through products exactly; PSUM accumulates fp32).
For every product A·B we compute Ahi·Bhi + Ahi·Blo + Alo·Bhi with
hi = trunc11(A), lo = A - hi; the dropped lo·lo term is O(2^-22) relative.
Measured on HW: a [128x128]@[128x256] split matmul lands at 1.9e-7 rel err
vs 1.7e-7 for native fp32.  This chaotic recurrence amplifies per-step noise
~3e5x, so plain f32r (1.7e-4/step) fails by ~100x while the split stays near
the fp32 envelope (tolerance 2e-2; numpy sim of this scheme: 1.8e-3, the
extra vs plain split-fp32r coming from round11(d) and the dropped Wclo*d).

Per step/core (batch slice BC=256), 22 matmuls all N=256 @1cyc/row:
  x-part  4: chunkA [v3hi2|xhi63|d|xlo63] K=128, chunkB [v3hi|xhi63|v3lo] K=65
  recur  12: (Wphi,hhi) (Wphi,hlo) (Wplo,hhi) x 2 K-chunks x 2 out-halves
  fc      6: (fchi,hhi) (fchi,hlo) (fclo,hhi) x 2 K-chunks, M=1
PE stream order per step: [mt0 block, mt1 block, fc] with each block
[K0hi K1hi Wlo0 Wlo1 Whi-hlo0 xB Whi-hlo1 xA(stop)] so hp0 closes early
(its tanh overlaps the rest of the step) and the d-row-dependent xA sits
last.  ACT: hhi (f32r round-on-write) + h per half, then the d-row copy and
the two v3hi rounding copies.  DVE: two hlo subtracts, the v stt, v3lo.
All engine writes land at 32-aligned base partitions (0/64), a TRN2
requirement.
"""

import os

os.environ.setdefault("MYCRO_LOCAL_CACHE", "1")

from contextlib import ExitStack

import numpy as np

import concourse.tile as tile
from concourse import bacc, mybir
from concourse.alu_op_type import AluOpType
from concourse.bass_utils import run_bass_kernel_spmd

T_FULL = 512
F = 63
H = 256
B_FULL = 2048
NCORES = 8
BC = B_FULL // NCORES  # 256 batch per core
CH = 8                 # time steps per x chunk tile
F32 = mybir.dt.float32
F32R = mybir.dt.float32r

KA = 2 * F + 2         # chunk A rows: v4hi2(1) + xhi(63) + d2(1) + xlo(63)
KB = 97                # chunk B rows: v4hi(1) + xhi(63) + v4lo(1) + pad + d3
RD = F + 1             # d2 row index in chunk A (= 64)
RD3 = 96               # d3 row index in chunk B

_CACHE: dict = {}


def _trunc11(x):
    u = np.ascontiguousarray(np.asarray(x, np.float32)).view(np.uint32)
    return (u & np.uint32(0xFFFFF000)).view(np.float32)


def _build(T: int):
    if T in _CACHE:
        return _CACHE[T]

    NSLOT = T + 4              # slot s holds step s+1's rows; +4 for v4 tail
    NCHUNK = (NSLOT + CH - 1) // CH
    nc = bacc.Bacc(
        "TRN2", target_bir_lowering=False, debug=False, enable_asserts=False
    )
    xAd = nc.dram_tensor("xA", [NCHUNK, KA, CH, BC], F32R, kind="ExternalInput").ap()
    xBd = nc.dram_tensor("xB", [NCHUNK, KB, CH, BC], F32R, kind="ExternalInput").ap()
    lhsAd = nc.dram_tensor("lhsA", [KA, 2, 128], F32R, kind="ExternalInput").ap()
    lhsBd = nc.dram_tensor("lhsB", [KB, 2, 128], F32R, kind="ExternalInput").ap()
    wphid = nc.dram_tensor("wphi", [128, 2, H], F32R, kind="ExternalInput").ap()
    wplod = nc.dram_tensor("wplo", [128, 2, H], F32R, kind="ExternalInput").ap()
    fchid = nc.dram_tensor("fchi", [128, 2], F32R, kind="ExternalInput").ap()
    fclod = nc.dram_tensor("fclo", [128, 2], F32R, kind="ExternalInput").ap()
    biasd = nc.dram_tensor("bias", [128, 2], F32, kind="ExternalInput").ap()
    fcbd = nc.dram_tensor("fcb", [1, 1], F32, kind="ExternalInput").ap()
    vind = nc.dram_tensor("vinit", [2, BC], F32, kind="ExternalInput").ap()
    voutd = nc.dram_tensor("vout", [T, 1, BC], F32, kind="ExternalOutput").ap()

    TANH = mybir.ActivationFunctionType.Tanh
    COPY = mybir.ActivationFunctionType.Copy
    SUB = AluOpType.subtract

    with tile.TileContext(nc) as tc, ExitStack() as ctx:
        consts = ctx.enter_context(tc.tile_pool(name="consts", bufs=1))
        lhsA = consts.tile([KA, 2, 128], F32R)
        lhsB = consts.tile([KB, 2, 128], F32R)
        wphi = consts.tile([128, 2, H], F32R)
        wplo = consts.tile([128, 2, H], F32R)
        fchi = consts.tile([128, 2], F32R)
        fclo = consts.tile([128, 2], F32R)
        bias = consts.tile([128, 2], F32)
        fcb = consts.tile([1, 1], F32)
        vin1 = consts.tile([1, BC], F32)
        nc.sync.dma_start(lhsA[:], lhsAd[:])
        nc.sync.dma_start(lhsB[:], lhsBd[:])
        nc.sync.dma_start(wphi[:], wphid[:])
        nc.sync.dma_start(wplo[:], wplod[:])
        nc.sync.dma_start(fchi[:], fchid[:])
        nc.sync.dma_start(fclo[:], fclod[:])
        nc.sync.dma_start(bias[:], biasd[:])
        nc.sync.dma_start(fcb[:], fcbd[:])
        nc.sync.dma_start(vin1[:], vind[1:2, :])

        xapool = ctx.enter_context(tc.tile_pool(name="xapool", bufs=4))
        xbpool = ctx.enter_context(tc.tile_pool(name="xbpool", bufs=4))
        vlpool = ctx.enter_context(tc.tile_pool(name="vlpool", bufs=4))
        hpool = ctx.enter_context(tc.tile_pool(name="hpool", bufs=3))
        hsplit = ctx.enter_context(tc.tile_pool(name="hsplit", bufs=3))
        ppool = ctx.enter_context(tc.tile_pool(name="ppool", bufs=3, space="PSUM"))
        dpool = ctx.enter_context(tc.tile_pool(name="dpool", bufs=2, space="PSUM"))

        xatiles: dict = {}
        xbtiles: dict = {}

        def xachunk(c):
            if c not in xatiles:
                xt = xapool.tile([KA, CH, BC], F32R, name="xa", tag="xa")
                if c == 0:
                    nc.sync.dma_start(xt[:], xAd[c])
                else:
                    # rows 1..127; row 64 (d) is zero in dram, overwritten by
                    # the per-slot ACT d-copy
                    nc.sync.dma_start(xt[1:KA], xAd[c, 1:KA])
                xatiles[c] = xt
            return xatiles[c]

        def xbchunk(c):
            if c not in xbtiles:
                xt = xbpool.tile([KB, CH, BC], F32R, name="xb", tag="xb")
                if c == 0:
                    nc.sync.dma_start(xt[:], xBd[c])
                else:
                    # rows 1..95: x + the zero pad band; rows 0/64/96 are
                    # engine-written per slot
                    nc.sync.dma_start(xt[1:RD3], xBd[c, 1:RD3])
                xbtiles[c] = xt
            return xbtiles[c]

        def slot_a(s):
            return xachunk(s // CH)[:, s % CH, :]

        def slot_b(s):
            return xbchunk(s // CH)[:, s % CH, :]

        def vrow_hi2(s):   # chunk A row 0 (pairs Wclo)
            return xachunk(s // CH)[0:1, s % CH, :]

        def drow(s):       # chunk A row 64: d2 (pairs -Wchi)
            return xachunk(s // CH)[RD:RD + 1, s % CH, :]

        def drow3(s):      # chunk B row 96: d3 (pairs -Wchi)
            return xbchunk(s // CH)[RD3:RD3 + 1, s % CH, :]

        def vrow_hi(s):    # chunk B row 0 (pairs Wchi)
            return xbchunk(s // CH)[0:1, s % CH, :]

        def vrow_lo(s):    # chunk B row 64 (pairs Wchi)
            return xbchunk(s // CH)[F + 1:F + 2, s % CH, :]

        h_prev = None      # (hhi, hlo) tiles of step t-1, layout [128, 2*BC]
        vf: dict = {}      # s -> [1, BC] fp32 tile with v_s

        def emit_fc(hhi, hlo, s):
            """d_s = fc . h_s (6 f32r matmuls into one PSUM row), then the
            rounded d row for slot s+1 (ACT) and the v chain for v_s:
            stt (DVE), v3hi/v3hi2 rounding copies (ACT) into slot s+3,
            v3lo subtract (DVE)."""
            d = dpool.tile([1, BC], F32, name="d", tag="d")
            nc.tensor.matmul(d[:], fchi[:, 0:1], hhi[:, 0:BC],
                             start=True, stop=False)
            nc.tensor.matmul(d[:], fchi[:, 1:2], hhi[:, BC:2 * BC],
                             start=False, stop=False)
            nc.tensor.matmul(d[:], fclo[:, 0:1], hhi[:, 0:BC],
                             start=False, stop=False)
            nc.tensor.matmul(d[:], fclo[:, 1:2], hhi[:, BC:2 * BC],
                             start=False, stop=False)
            nc.tensor.matmul(d[:], fchi[:, 0:1], hlo[:, 0:BC],
                             start=False, stop=False)
            nc.tensor.matmul(d[:], fchi[:, 1:2], hlo[:, BC:2 * BC],
                             start=False, stop=True)
            # d2 row for pre_{s+2} (1-step lag; Pool, keeping ACT/DVE
            # short) and d3 row for pre_{s+3} (2-step slack)
            if s + 1 <= T:
                nc.gpsimd.tensor_copy(drow(s + 1), d[:])
            if s + 2 <= T:
                nc.gpsimd.tensor_copy(drow3(s + 2), d[:])
            # v_s = (v_{s-1} - fcb) - d_s
            prev = vin1[:] if s == 1 else vf[s - 1][:]
            v = vlpool.tile([1, BC], F32, name="v", tag="v")
            nc.vector.scalar_tensor_tensor(
                v[:], prev, fcb[0:1, 0:1], d[:], op0=SUB, op1=SUB
            )
            vf[s] = v
            vf.pop(s - 2, None)
            nc.sync.dma_start(voutd[s - 1], v[:])
            # v4 rows for pre_{s+4}: three steps of slack (DVE + Pool)
            if s + 3 <= T:
                nc.vector.tensor_copy(vrow_hi(s + 3), v[:])
                nc.vector.tensor_tensor(
                    vrow_lo(s + 3), v[:], vrow_hi(s + 3).bitcast(F32), SUB
                )
                nc.gpsimd.tensor_copy(vrow_hi2(s + 3), v[:])

        for t in range(1, T + 1):
            hp = [
                ppool.tile([128, BC], F32, name="hp0", tag="hp0"),
                ppool.tile([128, BC], F32, name="hp1", tag="hp1"),
            ]
            first = h_prev is None
            ra = slot_a(t - 1)
            rb = slot_b(t - 1)
            h = hpool.tile([128, 2 * BC], F32, name="h", tag="h")
            hhi_n = hsplit.tile([128, 2 * BC], F32R, name="hhi", tag="hhi")
            hlo_n = hsplit.tile([128, 2 * BC], F32R, name="hlo", tag="hlo")
            if not first:
                hhi, hlo = h_prev
            if not first:
                # PE slot order: dependency-free xb opens both groups, each
                # operand class sits at a position late enough for its
                # producer chain (see docstring); hl1 closes each group
                def mm(key, mt):
                    ws = slice(mt * 128, (mt + 1) * 128)
                    return {
                        "k0hi": (wphi[:, 0, ws], hhi[:, 0:BC]),
                        "k1hi": (wphi[:, 1, ws], hhi[:, BC:2 * BC]),
                        "lo0": (wplo[:, 0, ws], hhi[:, 0:BC]),
                        "lo1": (wplo[:, 1, ws], hhi[:, BC:2 * BC]),
                        "hl0": (wphi[:, 0, ws], hlo[:, 0:BC]),
                        "hl1": (wphi[:, 1, ws], hlo[:, BC:2 * BC]),
                        "xa": (lhsA[:, mt, :], ra),
                        "xb": (lhsB[:, mt, :], rb),
                    }[key]
                slots = os.environ.get(
                    "KSLOTS",
                    "xb0,xb1,k0hi0,k0hi1,lo00,lo01,k1hi0,k1hi1,"
                    "xa0,xa1,hl00,hl01,lo10,hl10,lo11,hl11",
                ).split(",")
                seen: dict = {}
                for key in slots:
                    seen[int(key[-1])] = seen.get(int(key[-1]), 0) + 1
                cnt = {0: 0, 1: 0}
                for key in slots:
                    mt = int(key[-1])
                    w, r = mm(key[:-1], mt)
                    cnt[mt] += 1
                    nc.tensor.matmul(hp[mt], w, r, start=(cnt[mt] == 1),
                                     stop=(cnt[mt] == seen[mt]))
            else:
                for mt in range(2):
                    nc.tensor.matmul(hp[mt], lhsB[:, mt, :], rb,
                                     start=True, stop=False)
                    nc.tensor.matmul(hp[mt], lhsA[:, mt, :], ra,
                                     start=False, stop=True)
            # ACT: per half, f32r hhi (round-on-write) first — it feeds the
            # next step's lead matmuls — then fp32 h; DVE subtracts hlo
            for mt in range(2):
                sl = slice(mt * BC, (mt + 1) * BC)
                nc.scalar.activation(hhi_n[:, sl], hp[mt], TANH,
                                     bias=bias[:, mt:mt + 1])
                nc.scalar.activation(h[:, sl], hp[mt], TANH,
                                     bias=bias[:, mt:mt + 1])
                nc.vector.tensor_tensor(
                    hlo_n[:, sl], h[:, sl], hhi_n[:, sl].bitcast(F32), SUB
                )
            if not first:
                emit_fc(hhi, hlo, t - 1)
            h_prev = (hhi_n, hlo_n)

        # tail: fc + v chain for step T
        hhi, hlo = h_prev
        emit_fc(hhi, hlo, T)

    nc.compile()
    _CACHE[T] = nc
    return nc


def _prep_maps(x_seq, seed_capacity, W_ih_w, W_ih_b, W_hh_w, W_hh_b, fc_w, fc_b, T):
    x_seq = np.asarray(x_seq, dtype=np.float32)
    seed = np.asarray(seed_capacity, dtype=np.float32).reshape(B_FULL)
    W_ih_w = np.asarray(W_ih_w, dtype=np.float32)
    W_ih_b = np.asarray(W_ih_b, dtype=np.float32)
    W_hh_w = np.asarray(W_hh_w, dtype=np.float32)
    W_hh_b = np.asarray(W_hh_b, dtype=np.float32)
    fc_w = np.asarray(fc_w, dtype=np.float32)
    fc_b = np.asarray(fc_b, dtype=np.float32)

    Wx = W_ih_w[:, :F]            # [H, 63]
    Wc = W_ih_w[:, F]             # [H]
    bvec = W_ih_b + W_hh_b        # [H]
    fcb_val = float(fc_b[0])
    Wp = (W_hh_w - np.outer(Wc, fc_w[0])).astype(np.float32)
    fc = fc_w[0]

    WxT = np.ascontiguousarray(Wx.T)               # [63, H]
    WxTh = _trunc11(WxT); WxTl = (WxT - WxTh).astype(np.float32)
    Wch = _trunc11(Wc); Wcl = (Wc - Wch).astype(np.float32)
    WpTh = _trunc11(Wp.T); WpTl = (Wp.T - WpTh).astype(np.float32)
    fch = _trunc11(fc); fcl = (fc - fch).astype(np.float32)

    # lhsA [KA=128, 2, 128]: row 0 Wclo (v3hi2), 1..63 WxThi (xhi),
    # 64 -Wchi (d row), 65..127 WxThi (xlo)
    lhsA = np.zeros((KA, H), np.float32)
    lhsA[0] = Wcl
    lhsA[1:F + 1] = WxTh
    lhsA[RD] = -Wch
    lhsA[RD + 1:] = WxTh
    lhsA = np.ascontiguousarray(lhsA.reshape(KA, 2, 128))
    # lhsB [KB=97, 2, 128]: row 0 Wchi (v4hi), 1..63 WxTlo (xhi), 64 Wchi
    # (v4lo), 65..95 zero pad, 96 -Wchi (d3)
    lhsB = np.zeros((KB, H), np.float32)
    lhsB[0] = Wch
    lhsB[1:F + 1] = WxTl
    lhsB[F + 1] = Wch
    lhsB[RD3] = -Wch
    lhsB = np.ascontiguousarray(lhsB.reshape(KB, 2, 128))

    wphi = np.ascontiguousarray(WpTh.reshape(2, 128, H).transpose(1, 0, 2))
    wplo = np.ascontiguousarray(WpTl.reshape(2, 128, H).transpose(1, 0, 2))
    fchi = np.ascontiguousarray(fch.reshape(2, 128).T)      # [128, 2]
    fclo = np.ascontiguousarray(fcl.reshape(2, 128).T)
    # bias folds in the -2*Wc*fcb constant from
    # v_{t-2} = v_{t-4} - 2*fcb - d_{t-3} - d_{t-2}
    biasm = np.ascontiguousarray(
        (bvec - 2 * Wc * fcb_val).astype(np.float32).reshape(2, 128).T)
    fcb = np.array([[fcb_val]], dtype=np.float32)

    NSLOT = T + 4
    NCHUNK = (NSLOT + CH - 1) // CH

    in_maps = []
    for c in range(NCORES):
        sl = slice(c * BC, (c + 1) * BC)
        xc = x_seq[sl, :T, :]                                # [BC, T, F]
        xtr = np.ascontiguousarray(xc.transpose(1, 2, 0))    # [T, F, BC]
        Tp = NCHUNK * CH
        xtr = np.concatenate(
            [xtr, np.zeros((Tp - T, F, BC), np.float32)], axis=0
        )
        xch = xtr.reshape(NCHUNK, CH, F, BC).transpose(0, 2, 1, 3)
        xch_hi = _trunc11(xch)
        xch_lo = (xch - xch_hi).astype(np.float32)
        xA = np.zeros((NCHUNK, KA, CH, BC), np.float32)
        xA[:, 1:F + 1] = xch_hi
        xA[:, RD + 1:] = xch_lo
        xB = np.zeros((NCHUNK, KB, CH, BC), np.float32)
        xB[:, 1:F + 1] = xch_hi
        seedc = seed[sl]                                     # cap_0 = v_{-1}
        v0 = (seedc - fcb_val).astype(np.float32)
        # slot s carries v4 = v_{s-3}; early slots use fcb-shifted v_{-1}
        # with d rows 0 so that v4 - 2*fcb - d3 - d2 reproduces v_{t-2}
        for slot, vval in ((0, (seedc + 2 * fcb_val).astype(np.float32)),
                           (1, (seedc + fcb_val).astype(np.float32)),
                           (2, seedc), (3, v0)):
            vh = _trunc11(vval)
            xB[0, 0, slot] = vh                              # v4hi
            xB[0, F + 1, slot] = vval - vh                   # v4lo
            xA[0, 0, slot] = vh                              # v4hi2
        in_maps.append(
            {
                "xA": np.ascontiguousarray(xA),
                "xB": np.ascontiguousarray(xB),
                "lhsA": lhsA,
                "lhsB": lhsB,
                "wphi": wphi,
                "wplo": wplo,
                "fchi": fchi,
                "fclo": fclo,
                "bias": biasm,
                "fcb": fcb,
                "vinit": np.ascontiguousarray(np.stack([seedc, v0])),
            }
        )
    return in_maps, fcb_val


def _run(trace=False, **inputs):
    T = int(inputs.get("forecast_steps", T_FULL))
    nc = _build(T)
    in_maps, fcb_val = _prep_maps(
        inputs["x_seq"], inputs["seed_capacity"],
        inputs["W_ih_w"], inputs["W_ih_b"],
        inputs["W_hh_w"], inputs["W_hh_b"],
        inputs["fc_w"], inputs["fc_b"], T,
    )
    res = run_bass_kernel_spmd(
        nc, in_maps, core_ids=list(range(NCORES)), trace=trace
    )
    out = np.empty((B_FULL, T), np.float32)
    for c in range(NCORES):
        v = res.results[c]["vout"].reshape(T, BC)
        out[c * BC:(c + 1) * BC] = (v + fcb_val).T
    return out, res


def kernel(**inputs) -> np.ndarray:
    out, _ = _run(trace=False, **inputs)
    return out
